# revision 3
# baseline (speedup 1.0000x reference)
"""Trainium2 Bass kernel: 2-layer GRU (H=200) + fc/tanh head, teacher-forced inputs.

Architecture (per NeuronCore, data-parallel over batch, 16 batch rows/core):
  - Layout: "H-major" — hidden/gate dims on SBUF partitions, batch on the free dim.
  - Gate pre-activations gh = W_hh @ h + b_hh computed per step as 12 small
    matmuls (6 gate-chunks of 100 x 2 K-chunks of ~100); biases folded in via a
    constant ones-row appended to the hidden state (K=101 for chunk 0).
  - Input projections gx0 (from x) and gx1 (from h0) are computed as batched
    chunk-GEMMs (32 timesteps at a time, N=512) off the recurrence critical path.
  - h0 history lives in an SBUF ring (5 chunks) feeding the gx1 chunk-GEMM;
    layer-1 scan runs one chunk behind layer-0, interleaved cell-by-cell so all
    engines stay busy.
  - fc output (4 x 16 per step) accumulates into one PSUM bank per chunk; a
    single tanh over [4, 512] flushes it to SBUF and DMA to HBM.
"""

import numpy as np

import concourse.bacc as bacc
import concourse.mybir as mybir
import concourse.tile as tile
from concourse import bass_utils

F32 = mybir.dt.float32
AF = mybir.ActivationFunctionType

B = 128          # full batch
T = 1024         # timesteps
H = 200          # hidden size
HC = 100         # hidden chunk (2 chunks per H)
G3 = 3 * H       # 600 gate rows
NG = 6           # gate chunks of HC
IN0 = 8          # layer-0 input size
OUT = 4          # fc output size
NCORES = 8
BC = B // NCORES  # 16 batch rows per core
CH = 32          # timesteps per chunk
RING = 5         # h0 history ring depth (chunks)


def _build_nc(t_steps=T, ch=CH):
    nchunk = t_steps // ch
    nc = bacc.Bacc("TRN2", target_bir_lowering=False, debug=False)

    x9 = nc.dram_tensor("x9", (IN0 + 1, t_steps * BC), F32, kind="ExternalInput")
    w0 = nc.dram_tensor("w0", (IN0 + 1, G3), F32, kind="ExternalInput")
    whh0a = nc.dram_tensor("whh0a", (HC + 1, G3), F32, kind="ExternalInput")
    whh0b = nc.dram_tensor("whh0b", (HC, G3), F32, kind="ExternalInput")
    wih1a = nc.dram_tensor("wih1a", (HC + 1, G3), F32, kind="ExternalInput")
    wih1b = nc.dram_tensor("wih1b", (HC, G3), F32, kind="ExternalInput")
    whh1a = nc.dram_tensor("whh1a", (HC + 1, G3), F32, kind="ExternalInput")
    whh1b = nc.dram_tensor("whh1b", (HC, G3), F32, kind="ExternalInput")
    wfca = nc.dram_tensor("wfca", (HC + 1, OUT), F32, kind="ExternalInput")
    wfcb = nc.dram_tensor("wfcb", (HC, OUT), F32, kind="ExternalInput")
    yt = nc.dram_tensor("yt", (OUT, t_steps * BC), F32, kind="ExternalOutput")

    with tile.TileContext(nc) as tc:
        with (
            tc.tile_pool(name="persist", bufs=1) as persist,
            tc.tile_pool(name="x9p", bufs=2) as x9p,
            tc.tile_pool(name="gx0p", bufs=2) as gx0p,
            tc.tile_pool(name="gx1p", bufs=2) as gx1p,
            tc.tile_pool(name="outp", bufs=2) as outp,
            tc.tile_pool(name="elt", bufs=3) as elt,
            tc.tile_pool(name="ps_gx0", bufs=1, space="PSUM") as ps_gx0,
            tc.tile_pool(name="ps_gx1", bufs=2, space="PSUM") as ps_gx1,
            tc.tile_pool(name="ps_l0", bufs=2, space="PSUM") as ps_l0,
            tc.tile_pool(name="ps_l1", bufs=2, space="PSUM") as ps_l1,
            tc.tile_pool(name="ps_fc", bufs=1, space="PSUM") as ps_fc,
        ):
            # ---- persistent SBUF tiles ----
            w0sb = persist.tile([IN0 + 1, G3], F32, tag="w0sb")
            whh0a_s = persist.tile([HC + 1, G3], F32, tag="whh0a")
            whh0b_s = persist.tile([HC, G3], F32, tag="whh0b")
            wih1a_s = persist.tile([HC + 1, G3], F32, tag="wih1a")
            wih1b_s = persist.tile([HC, G3], F32, tag="wih1b")
            whh1a_s = persist.tile([HC + 1, G3], F32, tag="whh1a")
            whh1b_s = persist.tile([HC, G3], F32, tag="whh1b")
            wfca_s = persist.tile([HC + 1, OUT], F32, tag="wfca")
            wfcb_s = persist.tile([HC, OUT], F32, tag="wfcb")
            # h0 ring: [101, ring-chunk, step, (k,b)]
            ring = persist.tile([HC + 1, RING, ch, 2 * BC], F32, tag="ring")
            h1t = persist.tile([HC + 1, 2, 2 * BC], F32, tag="h1t")
            z0 = persist.tile([HC + 1, 2 * BC], F32, tag="z0")

            for dst, src in (
                (w0sb, x9), (w0sb, w0), (whh0a_s, whh0a), (whh0b_s, whh0b),
                (wih1a_s, wih1a), (wih1b_s, wih1b), (whh1a_s, whh1a),
                (whh1b_s, whh1b), (wfca_s, wfca), (wfcb_s, wfcb),
            ):
                if src is x9:
                    continue
                nc.sync.dma_start(dst[:], src[:])

            # ones-rows (partition 100) can't be memset directly (base must be
            # quadrant-aligned): set whole tile to 1.0 then zero rows 0:100.
            nc.gpsimd.memset(z0[:], 1.0)
            nc.gpsimd.memset(z0[0:HC, :], 0.0)
            nc.gpsimd.memset(ring[:], 1.0)
            nc.gpsimd.memset(h1t[:], 1.0)
            nc.gpsimd.memset(h1t[0:HC, 0, :], 0.0)

            gx0_tiles = {}
            gx1_tiles = {}
            fc_tiles = {}

            def ring_slot(t):
                c, j = divmod(t, ch)
                return ring[:, c % RING, j]  # AP [101, 32]

            def gx0_chunk(i):
                x9t = x9p.tile([IN0 + 1, ch * BC], F32, tag="x9t")
                nc.sync.dma_start(x9t[:], x9[:, i * ch * BC:(i + 1) * ch * BC])
                gxt = gx0p.tile([HC, ch, NG, BC], F32, tag="gx0t")
                gx0_tiles[i] = gxt
                for g in range(NG):
                    pq = ps_gx0.tile([HC, ch * BC], F32, tag="q0")
                    nc.tensor.matmul(pq[:], w0sb[:, g * HC:(g + 1) * HC], x9t[:],
                                     start=True, stop=True)
                    nc.scalar.copy(gxt[:, :, g, :], pq[:])

            def gx1_chunk(i):
                rc = ring[:, i % RING]  # [101, 32, 32]
                gxt = gx1p.tile([HC, ch, NG, BC], F32, tag="gx1t")
                gx1_tiles[i] = gxt
                for g in range(NG):
                    pq = ps_gx1.tile([HC, ch * BC], F32, tag="q1")
                    nc.tensor.matmul(pq[:], wih1a_s[:, g * HC:(g + 1) * HC],
                                     rc[0:HC + 1, :, 0:BC], start=True, stop=False)
                    nc.tensor.matmul(pq[:], wih1b_s[:, g * HC:(g + 1) * HC],
                                     rc[0:HC, :, BC:2 * BC], start=False, stop=True)
                    nc.vector.tensor_copy(gxt[:, :, g, :], pq[:])

            def gru_cell(t, prev, cur, gxt, j, wa, wb, ps_pool, ps_tag, tg):
                """One GRU cell in H-major layout. prev/cur: AP [101, 32]."""
                pg = ps_pool.tile([HC, NG * BC], F32, tag=ps_tag)
                for g in range(NG):
                    o = pg[:, g * BC:(g + 1) * BC]
                    nc.tensor.matmul(o, wa[:, g * HC:(g + 1) * HC],
                                     prev[0:HC + 1, 0:BC], start=True, stop=False)
                    nc.tensor.matmul(o, wb[:, g * HC:(g + 1) * HC],
                                     prev[0:HC, BC:2 * BC], start=False, stop=True)
                gsl = gxt[:, j]  # [100, 6, 16]
                s = elt.tile([HC, 4 * BC], F32, tag="s" + tg)
                nc.vector.tensor_add(s[:], pg[:, 0:4 * BC], gsl[:, 0:4, :])
                rz = elt.tile([HC, 4 * BC], F32, tag="rz" + tg)
                nc.scalar.activation(rz[:], s[:], AF.Sigmoid)
                tn = elt.tile([HC, 2 * BC], F32, tag="tn" + tg)
                nc.vector.tensor_mul(tn[:], rz[:, 0:2 * BC], pg[:, 4 * BC:6 * BC])
                np_ = elt.tile([HC, 2 * BC], F32, tag="np" + tg)
                nc.vector.tensor_add(np_[:], tn[:], gsl[:, 4:6, :])
                n_ = elt.tile([HC, 2 * BC], F32, tag="n" + tg)
                nc.scalar.activation(n_[:], np_[:], AF.Tanh)
                d = elt.tile([HC, 2 * BC], F32, tag="d" + tg)
                nc.vector.tensor_sub(d[:], prev[0:HC, :], n_[:])
                e = elt.tile([HC, 2 * BC], F32, tag="e" + tg)
                nc.vector.tensor_mul(e[:], rz[:, 2 * BC:4 * BC], d[:])
                nc.vector.tensor_add(cur[0:HC, :], e[:], n_[:])

            def l0_cell(t):
                i, j = divmod(t, ch)
                prev = z0[:] if t == 0 else ring_slot(t - 1)
                gru_cell(t, prev, ring_slot(t), gx0_tiles[i], j,
                         whh0a_s, whh0b_s, ps_l0, "l0g", "0")

            def l1_cell(t):
                i, j = divmod(t, ch)
                prev = h1t[:, t % 2]
                cur = h1t[:, (t + 1) % 2]
                gru_cell(t, prev, cur, gx1_tiles[i], j,
                         whh1a_s, whh1b_s, ps_l1, "l1g", "1")
                if j == 0:
                    fc_tiles[i] = ps_fc.tile([OUT, ch * BC], F32, tag="fc",
                                             name="fct")
                fcp = fc_tiles[i]
                o = fcp[:, j * BC:(j + 1) * BC]
                nc.tensor.matmul(o, wfca_s[:], cur[0:HC + 1, 0:BC],
                                 start=True, stop=False)
                nc.tensor.matmul(o, wfcb_s[:], cur[0:HC, BC:2 * BC],
                                 start=False, stop=True)

            def fc_flush(i):
                ot = outp.tile([OUT, ch * BC], F32, tag="ot")
                nc.scalar.activation(ot[:], fc_tiles[i][:], AF.Tanh)
                nc.sync.dma_start(yt[:, i * ch * BC:(i + 1) * ch * BC], ot[:])
                del fc_tiles[i]

            # ---- main pipelined loop ----
            gx0_chunk(0)
            for i in range(nchunk):
                if i >= 1:
                    gx1_chunk(i - 1)
                for j in range(ch):
                    l0_cell(i * ch + j)
                    if i >= 1:
                        l1_cell((i - 1) * ch + j)
                if i >= 1:
                    fc_flush(i - 1)
                if i + 1 < nchunk:
                    gx0_chunk(i + 1)
            gx1_chunk(nchunk - 1)
            for j in range(ch):
                l1_cell((nchunk - 1) * ch + j)
            fc_flush(nchunk - 1)

    nc.compile()
    return nc


_NC_CACHE = {}


def _get_nc(t_steps=T, ch=CH):
    key = (t_steps, ch)
    if key not in _NC_CACHE:
        _NC_CACHE[key] = _build_nc(t_steps, ch)
    return _NC_CACHE[key]


def _prep_weights(W_ih0, W_hh0, b_ih0, b_hh0, W_ih1, W_hh1, b_ih1, b_hh1,
                  W_fc, b_fc):
    f = lambda a: np.ascontiguousarray(np.asarray(a, np.float32))
    W_ih0, W_hh0, W_ih1, W_hh1, W_fc = map(f, (W_ih0, W_hh0, W_ih1, W_hh1, W_fc))
    b_ih0, b_hh0, b_ih1, b_hh1, b_fc = map(f, (b_ih0, b_hh0, b_ih1, b_hh1, b_fc))
    cat = lambda w, bias: np.ascontiguousarray(
        np.concatenate([w[:, :HC].T, bias[None, :]], axis=0), np.float32)
    return {
        "w0": np.ascontiguousarray(
            np.concatenate([W_ih0.T, b_ih0[None, :]], axis=0), np.float32),
        "whh0a": cat(W_hh0, b_hh0),
        "whh0b": np.ascontiguousarray(W_hh0[:, HC:].T),
        "wih1a": cat(W_ih1, b_ih1),
        "wih1b": np.ascontiguousarray(W_ih1[:, HC:].T),
        "whh1a": cat(W_hh1, b_hh1),
        "whh1b": np.ascontiguousarray(W_hh1[:, HC:].T),
        "wfca": cat(W_fc, b_fc),
        "wfcb": np.ascontiguousarray(W_fc[:, HC:].T),
    }


def _run(x, weights, t_steps=T, ch=CH, trace=False):
    """x: (B, t_steps, 8) float32 teacher-forcing raw input (as in reference)."""
    nc = _get_nc(t_steps, ch)
    bsz = x.shape[0]
    emotion = x[:, 0, 4:8]
    tf = np.concatenate([np.ones((bsz, 1, 4), np.float32), x[:, :-1, 0:4]], axis=1)
    inputs = np.concatenate(
        [tf, np.broadcast_to(emotion[:, None, :], (bsz, t_steps, 4))], axis=-1)

    in_maps = []
    for c in range(NCORES):
        xs = inputs[c * BC:(c + 1) * BC]  # [16, t, 8]
        x9 = np.empty((IN0 + 1, t_steps * BC), np.float32)
        x9[0:IN0] = xs.transpose(2, 1, 0).reshape(IN0, t_steps * BC)
        x9[IN0] = 1.0
        m = dict(weights)
        m["x9"] = x9
        in_maps.append(m)

    res = bass_utils.run_bass_kernel_spmd(
        nc, in_maps, core_ids=list(range(NCORES)), trace=trace)
    outs = [np.transpose(r["yt"].reshape(OUT, t_steps, BC), (2, 1, 0))
            for r in res.results]
    return np.concatenate(outs, axis=0), res


def kernel(x, W_ih0, W_hh0, b_ih0, b_hh0, W_ih1, W_hh1, b_ih1, b_hh1,
           W_fc, b_fc, xlens):
    x = np.ascontiguousarray(np.asarray(x, np.float32))
    weights = _prep_weights(W_ih0, W_hh0, b_ih0, b_hh0, W_ih1, W_hh1,
                            b_ih1, b_hh1, W_fc, b_fc)
    out, _ = _run(x, weights, T, CH)
    return out


# revision 7
# speedup vs baseline: 20.1889x; 20.1889x over previous
"""Trainium2 Bass kernel: 2-layer GRU (H=200) + fc/tanh head, teacher-forced inputs.

Architecture (per NeuronCore, data-parallel over batch, 16 batch rows/core):
  - Layout: "H-major" — hidden/gate dims on SBUF partitions, batch on the free dim.
  - Gate pre-activations gh = W_hh @ h + b_hh computed per step as 12 small
    matmuls (6 gate-chunks of 100 x 2 K-chunks of ~100); biases folded in via a
    constant ones-row appended to the hidden state (K=101 for chunk 0).
  - Input projections gx0 (from x) and gx1 (from h0) are computed as batched
    chunk-GEMMs (32 timesteps at a time, N=512) off the recurrence critical path.
  - h0 history lives in an SBUF ring (5 chunks) feeding the gx1 chunk-GEMM;
    layer-1 scan runs one chunk behind layer-0, interleaved cell-by-cell so all
    engines stay busy.
  - fc output (4 x 16 per step) accumulates into one PSUM bank per chunk; a
    single tanh over [4, 512] flushes it to SBUF and DMA to HBM.
"""

import numpy as np

import concourse.bacc as bacc
import concourse.mybir as mybir
import concourse.tile as tile
from concourse import bass_utils

F32 = mybir.dt.float32
AF = mybir.ActivationFunctionType

B = 128          # full batch
T = 1024         # timesteps
H = 200          # hidden size
HC = 100         # hidden chunk (2 chunks per H)
G3 = 3 * H       # 600 gate rows
NG = 6           # gate chunks of HC
IN0 = 8          # layer-0 input size
OUT = 4          # fc output size
NCORES = 8
BC = B // NCORES  # 16 batch rows per core
CH = 32          # timesteps per chunk
RING = 5         # h0 history ring depth (chunks)


def _build_nc(t_steps=T, ch=CH):
    nchunk = t_steps // ch
    nc = bacc.Bacc("TRN2", target_bir_lowering=False, debug=False)

    x9 = nc.dram_tensor("x9", (IN0 + 1, t_steps * BC), F32, kind="ExternalInput")
    w0 = nc.dram_tensor("w0", (IN0 + 1, G3), F32, kind="ExternalInput")
    whh0a = nc.dram_tensor("whh0a", (HC + 1, G3), F32, kind="ExternalInput")
    whh0b = nc.dram_tensor("whh0b", (HC, G3), F32, kind="ExternalInput")
    wih1a = nc.dram_tensor("wih1a", (HC + 1, G3), F32, kind="ExternalInput")
    wih1b = nc.dram_tensor("wih1b", (HC, G3), F32, kind="ExternalInput")
    whh1a = nc.dram_tensor("whh1a", (HC + 1, G3), F32, kind="ExternalInput")
    whh1b = nc.dram_tensor("whh1b", (HC, G3), F32, kind="ExternalInput")
    wfca = nc.dram_tensor("wfca", (HC + 1, OUT), F32, kind="ExternalInput")
    wfcb = nc.dram_tensor("wfcb", (HC, OUT), F32, kind="ExternalInput")
    yt = nc.dram_tensor("yt", (OUT, t_steps * BC), F32, kind="ExternalOutput")

    with tile.TileContext(nc) as tc:
        with (
            tc.tile_pool(name="persist", bufs=1) as persist,
            tc.tile_pool(name="x9p", bufs=2) as x9p,
            tc.tile_pool(name="gx0p", bufs=2) as gx0p,
            tc.tile_pool(name="gx1p", bufs=2) as gx1p,
            tc.tile_pool(name="outp", bufs=2) as outp,
            tc.tile_pool(name="elt", bufs=3) as elt,
            tc.tile_pool(name="ps_gx0", bufs=1, space="PSUM") as ps_gx0,
            tc.tile_pool(name="ps_gx1", bufs=2, space="PSUM") as ps_gx1,
            tc.tile_pool(name="ps_l0", bufs=2, space="PSUM") as ps_l0,
            tc.tile_pool(name="ps_l1", bufs=2, space="PSUM") as ps_l1,
            tc.tile_pool(name="ps_fc", bufs=1, space="PSUM") as ps_fc,
        ):
            # ---- persistent SBUF tiles ----
            w0sb = persist.tile([IN0 + 1, G3], F32, tag="w0sb")
            whh0a_s = persist.tile([HC + 1, G3], F32, tag="whh0a")
            whh0b_s = persist.tile([HC, G3], F32, tag="whh0b")
            wih1a_s = persist.tile([HC + 1, G3], F32, tag="wih1a")
            wih1b_s = persist.tile([HC, G3], F32, tag="wih1b")
            whh1a_s = persist.tile([HC + 1, G3], F32, tag="whh1a")
            whh1b_s = persist.tile([HC, G3], F32, tag="whh1b")
            wfca_s = persist.tile([HC + 1, OUT], F32, tag="wfca")
            wfcb_s = persist.tile([HC, OUT], F32, tag="wfcb")
            # h0 ring: [101, ring-chunk, step, (k,b)]
            ring = persist.tile([HC + 1, RING, ch, 2 * BC], F32, tag="ring")
            h1t = persist.tile([HC + 1, 2, 2 * BC], F32, tag="h1t")
            z0 = persist.tile([HC + 1, 2 * BC], F32, tag="z0")

            for dst, src in (
                (w0sb, x9), (w0sb, w0), (whh0a_s, whh0a), (whh0b_s, whh0b),
                (wih1a_s, wih1a), (wih1b_s, wih1b), (whh1a_s, whh1a),
                (whh1b_s, whh1b), (wfca_s, wfca), (wfcb_s, wfcb),
            ):
                if src is x9:
                    continue
                nc.sync.dma_start(dst[:], src[:])

            # ones-rows (partition 100) can't be memset directly (base must be
            # quadrant-aligned): set whole tile to 1.0 then zero rows 0:100.
            nc.gpsimd.memset(z0[:], 1.0)
            nc.gpsimd.memset(z0[0:HC, :], 0.0)
            nc.gpsimd.memset(ring[:], 1.0)
            nc.gpsimd.memset(h1t[:], 1.0)
            nc.gpsimd.memset(h1t[0:HC, 0, :], 0.0)

            gx0_tiles = {}
            gx1_tiles = {}
            fc_tiles = {}

            def ring_slot(t):
                c, j = divmod(t, ch)
                return ring[:, c % RING, j]  # AP [101, 32]

            def gx0_chunk(i):
                x9t = x9p.tile([IN0 + 1, ch * BC], F32, tag="x9t")
                nc.sync.dma_start(x9t[:], x9[:, i * ch * BC:(i + 1) * ch * BC])
                gxt = gx0p.tile([HC, ch, NG, BC], F32, tag="gx0t")
                gx0_tiles[i] = gxt
                for g in range(NG):
                    pq = ps_gx0.tile([HC, ch * BC], F32, tag="q0")
                    nc.tensor.matmul(pq[:], w0sb[:, g * HC:(g + 1) * HC], x9t[:],
                                     start=True, stop=True)
                    nc.scalar.copy(gxt[:, :, g, :], pq[:])

            def gx1_chunk(i):
                rc = ring[:, i % RING]  # [101, 32, 32]
                gxt = gx1p.tile([HC, ch, NG, BC], F32, tag="gx1t")
                gx1_tiles[i] = gxt
                for g in range(NG):
                    pq = ps_gx1.tile([HC, ch * BC], F32, tag="q1")
                    nc.tensor.matmul(pq[:], wih1a_s[:, g * HC:(g + 1) * HC],
                                     rc[0:HC + 1, :, 0:BC], start=True, stop=False)
                    nc.tensor.matmul(pq[:], wih1b_s[:, g * HC:(g + 1) * HC],
                                     rc[0:HC, :, BC:2 * BC], start=False, stop=True)
                    nc.vector.tensor_copy(gxt[:, :, g, :], pq[:])

            def gru_cell(t, prev, cur, gxt, j, wa, wb, ps_pool, ps_tag, tg):
                """One GRU cell in H-major layout. prev/cur: AP [101, 32]."""
                pg = ps_pool.tile([HC, NG * BC], F32, tag=ps_tag)
                for g in range(NG):
                    o = pg[:, g * BC:(g + 1) * BC]
                    nc.tensor.matmul(o, wa[:, g * HC:(g + 1) * HC],
                                     prev[0:HC + 1, 0:BC], start=True, stop=False)
                    nc.tensor.matmul(o, wb[:, g * HC:(g + 1) * HC],
                                     prev[0:HC, BC:2 * BC], start=False, stop=True)
                gsl = gxt[:, j]  # [100, 6, 16]
                s = elt.tile([HC, 4 * BC], F32, tag="s" + tg)
                nc.vector.tensor_add(s[:], pg[:, 0:4 * BC], gsl[:, 0:4, :])
                rz = elt.tile([HC, 4 * BC], F32, tag="rz" + tg)
                nc.scalar.activation(rz[:], s[:], AF.Sigmoid)
                tn = elt.tile([HC, 2 * BC], F32, tag="tn" + tg)
                nc.vector.tensor_mul(tn[:], rz[:, 0:2 * BC], pg[:, 4 * BC:6 * BC])
                np_ = elt.tile([HC, 2 * BC], F32, tag="np" + tg)
                nc.vector.tensor_add(np_[:], tn[:], gsl[:, 4:6, :])
                n_ = elt.tile([HC, 2 * BC], F32, tag="n" + tg)
                nc.scalar.activation(n_[:], np_[:], AF.Tanh)
                d = elt.tile([HC, 2 * BC], F32, tag="d" + tg)
                nc.vector.tensor_sub(d[:], prev[0:HC, :], n_[:])
                e = elt.tile([HC, 2 * BC], F32, tag="e" + tg)
                nc.vector.tensor_mul(e[:], rz[:, 2 * BC:4 * BC], d[:])
                nc.vector.tensor_add(cur[0:HC, :], e[:], n_[:])

            def l0_cell(t):
                i, j = divmod(t, ch)
                prev = z0[:] if t == 0 else ring_slot(t - 1)
                gru_cell(t, prev, ring_slot(t), gx0_tiles[i], j,
                         whh0a_s, whh0b_s, ps_l0, "l0g", "0")

            def l1_cell(t):
                i, j = divmod(t, ch)
                prev = h1t[:, t % 2]
                cur = h1t[:, (t + 1) % 2]
                gru_cell(t, prev, cur, gx1_tiles[i], j,
                         whh1a_s, whh1b_s, ps_l1, "l1g", "1")
                if j == 0:
                    fc_tiles[i] = ps_fc.tile([OUT, ch * BC], F32, tag="fc",
                                             name="fct")
                fcp = fc_tiles[i]
                o = fcp[:, j * BC:(j + 1) * BC]
                nc.tensor.matmul(o, wfca_s[:], cur[0:HC + 1, 0:BC],
                                 start=True, stop=False)
                nc.tensor.matmul(o, wfcb_s[:], cur[0:HC, BC:2 * BC],
                                 start=False, stop=True)

            def fc_flush(i):
                ot = outp.tile([OUT, ch * BC], F32, tag="ot")
                nc.scalar.activation(ot[:], fc_tiles[i][:], AF.Tanh)
                nc.sync.dma_start(yt[:, i * ch * BC:(i + 1) * ch * BC], ot[:])
                del fc_tiles[i]

            # ---- main pipelined loop ----
            gx0_chunk(0)
            for i in range(nchunk):
                if i >= 1:
                    gx1_chunk(i - 1)
                for j in range(ch):
                    l0_cell(i * ch + j)
                    if i >= 1:
                        l1_cell((i - 1) * ch + j)
                if i >= 1:
                    fc_flush(i - 1)
                if i + 1 < nchunk:
                    gx0_chunk(i + 1)
            gx1_chunk(nchunk - 1)
            for j in range(ch):
                l1_cell((nchunk - 1) * ch + j)
            fc_flush(nchunk - 1)

    nc.compile()
    return nc


_NC_CACHE = {}


def _get_nc(t_steps=T, ch=CH):
    key = (t_steps, ch)
    if key not in _NC_CACHE:
        _NC_CACHE[key] = _build_nc(t_steps, ch)
    return _NC_CACHE[key]


_RUNNER_CACHE = {}


def _get_runner(t_steps=T, ch=CH):
    """Build (once) a cached jit'd SPMD executable for the compiled Bass module.

    Mirrors concourse.bass2jax.run_bass_via_pjrt but caches the jitted
    callable so repeated invocations don't retrace/recompile.
    """
    key = (t_steps, ch)
    if key in _RUNNER_CACHE:
        return _RUNNER_CACHE[key]

    import jax
    from jax.sharding import Mesh, PartitionSpec
    from jax.experimental.shard_map import shard_map
    from concourse import bass2jax
    import concourse.mybir as _mybir

    nc = _get_nc(t_steps, ch)
    bass2jax.install_neuronx_cc_hook()
    assert nc.dbg_addr is None
    pid_name = nc.partition_id_tensor.name if nc.partition_id_tensor else None

    in_names, out_names, out_avals = [], [], []
    for alloc in nc.m.functions[0].allocations:
        if not isinstance(alloc, _mybir.MemoryLocationSet):
            continue
        name = alloc.memorylocations[0].name
        if alloc.kind == "ExternalInput":
            if name != pid_name:
                in_names.append(name)
        elif alloc.kind == "ExternalOutput":
            out_names.append(name)
            out_avals.append(jax.core.ShapedArray(
                tuple(alloc.tensor_shape), _mybir.dt.np(alloc.dtype)))
    n_params = len(in_names)
    all_names = in_names + out_names
    if pid_name is not None:
        all_names = all_names + [pid_name]
    donate = tuple(range(n_params, n_params + len(out_names)))

    def _body(*args):
        operands = list(args)
        if pid_name is not None:
            operands.append(bass2jax.partition_id_tensor())
        outs = bass2jax._bass_exec_p.bind(
            *operands,
            out_avals=tuple(out_avals),
            in_names=tuple(all_names),
            out_names=tuple(out_names),
            lowering_input_output_aliases=(),
            sim_require_finite=True,
            sim_require_nnan=True,
            nc=nc,
        )
        return tuple(outs)

    devices = jax.devices()[:NCORES]
    mesh = Mesh(np.asarray(devices), ("core",))
    in_specs = (PartitionSpec("core"),) * (n_params + len(out_names))
    out_specs = (PartitionSpec("core"),) * len(out_names)
    sharded = jax.jit(
        shard_map(_body, mesh=mesh, in_specs=in_specs, out_specs=out_specs,
                  check_rep=False),
        donate_argnums=donate, keep_unused=True)
    runner = (sharded, in_names, out_names, out_avals)
    _RUNNER_CACHE[key] = runner
    return runner


def _exec(in_maps, t_steps=T, ch=CH):
    """Run the cached executable on 8 cores; returns list of per-core out dicts."""
    sharded, in_names, out_names, out_avals = _get_runner(t_steps, ch)
    concat_in = [np.concatenate([m[name] for m in in_maps], axis=0)
                 for name in in_names]
    concat_zeros = [np.zeros((NCORES * a.shape[0], *a.shape[1:]), a.dtype)
                    for a in out_avals]
    out_arrs = sharded(*concat_in, *concat_zeros)
    out_arrs = [np.asarray(o) for o in out_arrs]
    return [
        {name: out_arrs[i].reshape(NCORES, *out_avals[i].shape)[c]
         for i, name in enumerate(out_names)}
        for c in range(NCORES)
    ]


def _prep_weights(W_ih0, W_hh0, b_ih0, b_hh0, W_ih1, W_hh1, b_ih1, b_hh1,
                  W_fc, b_fc):
    f = lambda a: np.ascontiguousarray(np.asarray(a, np.float32))
    W_ih0, W_hh0, W_ih1, W_hh1, W_fc = map(f, (W_ih0, W_hh0, W_ih1, W_hh1, W_fc))
    b_ih0, b_hh0, b_ih1, b_hh1, b_fc = map(f, (b_ih0, b_hh0, b_ih1, b_hh1, b_fc))
    cat = lambda w, bias: np.ascontiguousarray(
        np.concatenate([w[:, :HC].T, bias[None, :]], axis=0), np.float32)
    return {
        "w0": np.ascontiguousarray(
            np.concatenate([W_ih0.T, b_ih0[None, :]], axis=0), np.float32),
        "whh0a": cat(W_hh0, b_hh0),
        "whh0b": np.ascontiguousarray(W_hh0[:, HC:].T),
        "wih1a": cat(W_ih1, b_ih1),
        "wih1b": np.ascontiguousarray(W_ih1[:, HC:].T),
        "whh1a": cat(W_hh1, b_hh1),
        "whh1b": np.ascontiguousarray(W_hh1[:, HC:].T),
        "wfca": cat(W_fc, b_fc),
        "wfcb": np.ascontiguousarray(W_fc[:, HC:].T),
    }


def _make_in_maps(x, weights, t_steps=T):
    bsz = x.shape[0]
    emotion = x[:, 0, 4:8]
    tf = np.concatenate([np.ones((bsz, 1, 4), np.float32), x[:, :-1, 0:4]], axis=1)
    inputs = np.concatenate(
        [tf, np.broadcast_to(emotion[:, None, :], (bsz, t_steps, 4))], axis=-1)

    in_maps = []
    for c in range(NCORES):
        xs = inputs[c * BC:(c + 1) * BC]  # [16, t, 8]
        x9 = np.empty((IN0 + 1, t_steps * BC), np.float32)
        x9[0:IN0] = xs.transpose(2, 1, 0).reshape(IN0, t_steps * BC)
        x9[IN0] = 1.0
        m = dict(weights)
        m["x9"] = x9
        in_maps.append(m)
    return in_maps


def _run(x, weights, t_steps=T, ch=CH):
    """x: (B, t_steps, 8) float32 teacher-forcing raw input (as in reference)."""
    in_maps = _make_in_maps(x, weights, t_steps)
    results = _exec(in_maps, t_steps, ch)
    outs = [np.transpose(r["yt"].reshape(OUT, t_steps, BC), (2, 1, 0))
            for r in results]
    return np.concatenate(outs, axis=0), results


def kernel(x, W_ih0, W_hh0, b_ih0, b_hh0, W_ih1, W_hh1, b_ih1, b_hh1,
           W_fc, b_fc, xlens):
    x = np.ascontiguousarray(np.asarray(x, np.float32))
    weights = _prep_weights(W_ih0, W_hh0, b_ih0, b_hh0, W_ih1, W_hh1,
                            b_ih1, b_hh1, W_fc, b_fc)
    out, _ = _run(x, weights, T, CH)
    return out
